# revision 1
# baseline (speedup 1.0000x reference)
"""COIL-style sparse-attention scoring kernel for Trainium2 (8 NeuronCores).

Reference computation:
    scores[q,i,d,j] = <query_tok_embs[q,i], doc_tok_embs[d,j]>         (K=32)
    masked = where(query_ids[q,i]==doc_ids[d,j], scores, 0)
    tok    = masked.max(axis=j)                                        (192 -> 1)
    tok_scores[q,d] = sum_i w[q,i] * tok[q,i,d]    (w drops CLS + SEP)
    out = tok_scores + query_cls_emb @ doc_cls_emb.T

Device strategy (data-parallel over the 64 queries, 8 per core; doc side
replicated). The whole inner computation is ONE fp16 matmul per 512-column
PSUM bank plus a VectorE segmented max:

  * fp32 matmuls cost 4 cycles/column on TRN2, so the score matmul runs as
    an fp16 hi/lo 3-term decomposition at bf16 rate: q ~ qh + ql,
    d ~ dh + dl, score = qh.dh + qh.dl + ql.dh (error ~2^-22 relative).
  * Exact-match masking folds into the same contraction: token ids (< 7776)
    are encoded as base-6 digit quintuples -> 30 one-hot dims (0/1 doc side,
    C=128 query side; all exact in fp16). The combined K = 96 + 30 = 126
    matmul computes  aug = score + 128 * (#matching digits).  A full 5-digit
    match carries +640 while partial matches stay below 512 + |score|
    (|score| < 60 for this data, verified host-side), so
    relu(max_j(aug) - 640) == the reference where-masked max, exactly up to
    PSUM's 2^-14 rounding of the offset.
  * Segmented max over the 192 positions of each doc: VectorE tensor_reduce
    straight out of PSUM over [128, 8, 192] views of 3-bank groups (1536
    columns = exactly 8 docs).
  * decode relu(x-640) on ScalarE; per-token weights, the sum over query
    tokens, and the CLS scores fold into K=128 matmuls into one [8,128]
    PSUM tile.
"""

import numpy as np
from contextlib import ExitStack

import concourse.bass as bass
import concourse.bacc as bacc
import concourse.mybir as mybir
import concourse.tile as tile
from concourse.bass_utils import run_bass_kernel_spmd

F32 = mybir.dt.float32
F16 = mybir.dt.float16

# problem shape (hardcoded per contract)
BQ, LQ, BD, LD, TOK_D, CLS_D = 64, 32, 128, 192, 32, 768
NCORES = 8
QPC = BQ // NCORES          # 8 queries per core
NBLK = 2                    # two row-blocks of 128 = 4 queries x 32 tokens
ROWS = 128
DIG = 6                     # digit base; 6^5 = 7776 > 5000 vocab
NDIG = 5
KD = NDIG * DIG             # 30 one-hot dims
KS = 3 * TOK_D              # 96 = [qh; qh; ql] hi/lo score pack
KC = KS + KD                # 126 combined contraction
C = 128.0                   # per-digit match bonus
OFF = NDIG * C              # 640 full-match offset
ND = BD * LD                # 24576 doc positions
TN = 512                    # cols per matmul = one full PSUM bank
GRP = 3                     # PSUM banks per reduce group = 1536 cols = 8 docs
DGRP = GRP * TN // LD       # 8 docs per group
NG = ND // (GRP * TN)       # 16 groups per block
# rhs DMA chunk boundaries (small leading chunks so the PE starts sooner);
# multiples of 2048 so 512-col tiles never straddle
SBOUND = [0, 2048, 4096, 8192, 12288, 16384, 20480, ND]


def _chunk_of(bounds, col):
    for i in range(len(bounds) - 1):
        if bounds[i] <= col < bounds[i + 1]:
            return i, col - bounds[i]
    raise ValueError(col)


def build_nc():
    nc = bacc.Bacc(
        "TRN2",
        target_bir_lowering=False,
        debug=False,
        num_devices=NCORES,
    )

    qlhsT_d = nc.dram_tensor("qlhsT", [NBLK, KC, ROWS], F16, kind="ExternalInput")
    rhs_d = nc.dram_tensor("rhs", [KC, ND], F16, kind="ExternalInput")
    sel_d = nc.dram_tensor("sel", [ROWS, NBLK * QPC], F32, kind="ExternalInput")
    qclsT_d = nc.dram_tensor("qclsT", [CLS_D // 128, 128, QPC], F32, kind="ExternalInput")
    dclsT_d = nc.dram_tensor("dclsT", [CLS_D // 128, 128, BD], F32, kind="ExternalInput")
    out_d = nc.dram_tensor("out", [QPC, BD], F32, kind="ExternalOutput")

    with tile.TileContext(nc) as tc, ExitStack() as ctx:
        const = ctx.enter_context(tc.tile_pool(name="const", bufs=1))
        psum = ctx.enter_context(tc.tile_pool(name="psum", bufs=2, space="PSUM"))
        opsum = ctx.enter_context(tc.tile_pool(name="opsum", bufs=1, space="PSUM"))
        work = ctx.enter_context(tc.tile_pool(name="work", bufs=1))

        # --- load inputs; the big rhs is split over the three DMA-capable
        # engines (sync / gpsimd / scalar -> distinct queue sets) ---
        qclsT_t = const.tile([128, 6 * QPC], F32, tag="qclsT")
        dclsT_t = const.tile([128, 6 * BD], F32, tag="dclsT")
        qlhsT = const.tile([KC, NBLK * ROWS], F16, tag="qlhsT")
        sel_t = const.tile([ROWS, NBLK * QPC], F32, tag="sel")

        # the first score matmul gates the whole pipeline: its inputs (qlhsT
        # + rhs chunk 0) go first, with chunk 0 split across all three
        # queues by partition range
        for b in range(NBLK):
            nc.sync.dma_start(qlhsT[:, b * ROWS:(b + 1) * ROWS], qlhsT_d[b])
        rhs_tiles = []
        c1 = SBOUND[1]
        t0 = const.tile([KC, c1], F16, tag="rhs0")
        nc.sync.dma_start(t0[0:42, :], rhs_d[0:42, 0:c1])
        nc.gpsimd.dma_start(t0[42:84, :], rhs_d[42:84, 0:c1])
        nc.scalar.dma_start(t0[84:KC, :], rhs_d[84:KC, 0:c1])
        rhs_tiles.append(t0)
        engs = [nc.gpsimd, nc.scalar, nc.sync]
        for cch in range(1, len(SBOUND) - 1):
            c0, c1 = SBOUND[cch], SBOUND[cch + 1]
            t = const.tile([KC, c1 - c0], F16, tag=f"rhs{cch}")
            engs[cch % 3].dma_start(t[:], rhs_d[:, c0:c1])
            rhs_tiles.append(t)
        for k in range(6):
            nc.sync.dma_start(qclsT_t[:, k * QPC:(k + 1) * QPC], qclsT_d[k])
            nc.gpsimd.dma_start(dclsT_t[:, k * BD:(k + 1) * BD], dclsT_d[k])
        nc.scalar.dma_start(sel_t[:], sel_d[:])

        negoff_t = const.tile([128, 1], F32, tag="negoff")
        nc.gpsimd.memset(negoff_t[:], -OFF)

        out_ps = opsum.tile([QPC, BD], F32, tag="out_ps")

        # --- big combined matmuls + segmented max reduce ---
        tokdec = []
        for b in range(NBLK):
            tokred = work.tile([ROWS, BD], F32, tag=f"tokred{b}")
            lhs = qlhsT[:, b * ROWS:(b + 1) * ROWS]
            for g in range(NG):
                ps = psum.tile([128, GRP, TN], F32, tag="score")
                for k in range(GRP):
                    scol = (g * GRP + k) * TN
                    ci, off = _chunk_of(SBOUND, scol)
                    nc.tensor.matmul(
                        ps[:, k, :], lhs,
                        rhs_tiles[ci][:, off:off + TN],
                        start=True, stop=True,
                    )
                red_in = ps[:, :, :].rearrange("p g t -> p (g t)").rearrange(
                    "p (d j) -> p d j", j=LD
                )
                nc.vector.reduce_max(
                    tokred[:, DGRP * g:DGRP * (g + 1)],
                    red_in,
                    axis=mybir.AxisListType.X,
                )

            dec = work.tile([ROWS, BD], F32, tag=f"tokdec{b}")
            nc.scalar.activation(
                dec[:], tokred[:],
                mybir.ActivationFunctionType.Relu,
                bias=negoff_t[:], scale=1.0,
            )
            tokdec.append(dec)

        # --- final accumulation: CLS + weighted token sums (the CLS matmuls
        # have no score deps; the scheduler slots them into PE gaps) ---
        for k in range(6):
            nc.tensor.matmul(
                out_ps[:],
                qclsT_t[:, k * QPC:(k + 1) * QPC],
                dclsT_t[:, k * BD:(k + 1) * BD],
                start=(k == 0),
                stop=False,
            )
        for b in range(NBLK):
            nc.tensor.matmul(
                out_ps[:],
                sel_t[:, b * QPC:(b + 1) * QPC],
                tokdec[b][:],
                start=False,
                stop=(b == NBLK - 1),
            )

        outsb = work.tile([QPC, BD], F32, tag="outsb")
        nc.scalar.copy(outsb[:], out_ps[:])
        nc.sync.dma_start(out_d[:], outsb[:])

    nc.compile()
    return nc


_NC_CACHE = None


def _get_nc():
    global _NC_CACHE
    if _NC_CACHE is None:
        _NC_CACHE = build_nc()
    return _NC_CACHE


def _digit_onehot(ids, scale):
    """ids [...] int -> [..., 30] float32 one-hot of base-6 digits, scaled."""
    ids = ids.astype(np.int64)
    oh = np.zeros(ids.shape + (KD,), np.float32)
    flat = oh.reshape(-1, KD)
    fid = ids.reshape(-1)
    idx = np.arange(fid.size)
    for t in range(NDIG):
        flat[idx, t * DIG + (fid // (DIG ** t)) % DIG] = scale
    return oh


def _hilo(x):
    """fp32 array -> (hi, lo) float16 with x ~ hi + lo."""
    hi = x.astype(np.float16)
    lo = (x - hi.astype(np.float32)).astype(np.float16)
    return hi, lo


def make_in_maps(qte, dte, qce, dce, qid, did, qam):
    # SEP mask + CLS drop -> per-token weights
    sep = qam.sum(1) - 1
    qm = qam.astype(np.float32).copy()
    qm[np.arange(BQ), sep] = 0.0
    w = qm.copy()
    w[:, 0] = 0.0

    qoh = _digit_onehot(qid, C)                   # [64, 32, 30]
    doh = _digit_onehot(did, 1.0)                 # [128, 192, 30]

    dh, dl = _hilo(dte)                           # [128, 192, 32] fp16 each
    rhs = np.concatenate(
        [
            dh.transpose(2, 0, 1).reshape(TOK_D, ND),
            dl.transpose(2, 0, 1).reshape(TOK_D, ND),
            dh.transpose(2, 0, 1).reshape(TOK_D, ND),
            doh.transpose(2, 0, 1).reshape(KD, ND).astype(np.float16),
        ],
        axis=0,
    )  # [126, 24576] fp16: [dh; dl; dh; digit one-hots]
    dclsT = np.ascontiguousarray(dce.T.reshape(CLS_D // 128, 128, BD))

    in_maps = []
    for c in range(NCORES):
        qs = slice(c * QPC, (c + 1) * QPC)
        qte_c, qoh_c, w_c = qte[qs], qoh[qs], w[qs]

        qlhsT = np.zeros((NBLK, KC, ROWS), np.float16)
        for b in range(NBLK):
            blk = qte_c[b * 4:(b + 1) * 4].reshape(ROWS, TOK_D)
            qh, ql = _hilo(blk)
            qlhsT[b, 0:TOK_D] = qh.T            # pairs dh -> qh.dh
            qlhsT[b, TOK_D:2 * TOK_D] = qh.T    # pairs dl -> qh.dl
            qlhsT[b, 2 * TOK_D:KS] = ql.T       # pairs dh -> ql.dh
            qlhsT[b, KS:] = (
                qoh_c[b * 4:(b + 1) * 4].reshape(ROWS, KD).T.astype(np.float16)
            )

        sel = np.zeros((ROWS, NBLK * QPC), np.float32)
        for b in range(NBLK):
            for qq in range(4):
                ql_ = b * 4 + qq
                sel[qq * 32:(qq + 1) * 32, b * QPC + ql_] = w_c[ql_]

        qclsT = np.ascontiguousarray(qce[qs].T.reshape(CLS_D // 128, 128, QPC))

        in_maps.append(
            {
                "qlhsT": qlhsT,
                "rhs": np.ascontiguousarray(rhs),
                "sel": sel,
                "qclsT": qclsT,
                "dclsT": dclsT,
            }
        )
    return in_maps


def run(in_maps, trace=False, **kwargs):
    nc = _get_nc()
    return run_bass_kernel_spmd(
        nc, in_maps, core_ids=list(range(NCORES)), trace=trace, **kwargs
    )


def kernel(
    query_tok_embs,
    doc_tok_embs,
    query_cls_emb,
    doc_cls_emb,
    query_input_ids,
    doc_input_ids,
    query_attention_mask,
):
    qte = np.ascontiguousarray(np.asarray(query_tok_embs, np.float32))
    dte = np.ascontiguousarray(np.asarray(doc_tok_embs, np.float32))
    qce = np.ascontiguousarray(np.asarray(query_cls_emb, np.float32))
    dce = np.ascontiguousarray(np.asarray(doc_cls_emb, np.float32))
    qid = np.asarray(query_input_ids).astype(np.int64)
    did = np.asarray(doc_input_ids).astype(np.int64)
    qam = np.asarray(query_attention_mask).astype(np.int64)

    in_maps = make_in_maps(qte, dte, qce, dce, qid, did, qam)
    res = run(in_maps)
    out = np.concatenate([r["out"] for r in res.results], axis=0)
    return np.ascontiguousarray(out.astype(np.float32))



# revision 3
# speedup vs baseline: 3.6403x; 3.6403x over previous
"""COIL-style sparse-attention scoring kernel for Trainium2 (8 NeuronCores).

Reference computation:
    scores[q,i,d,j] = <query_tok_embs[q,i], doc_tok_embs[d,j]>         (K=32)
    masked = where(query_ids[q,i]==doc_ids[d,j], scores, 0)
    tok    = masked.max(axis=j)                                        (192 -> 1)
    tok_scores[q,d] = sum_i w[q,i] * tok[q,i,d]    (w drops CLS + SEP)
    out = tok_scores + query_cls_emb @ doc_cls_emb.T

Device strategy: data-parallel over the 64 queries (8 per core).  COIL is an
inverted-list model -- a doc position (d,j) can only contribute to a query
token with the *same* token id.  With |vocab|=5000 and 256 query tokens per
core, only ~5% of the 24576 doc positions match ANY of the core's query
tokens.  The host (integer id bookkeeping only -- all float math stays on
device) keeps just those columns, padded to CAP=24 slots per doc, shrinking
the score matmul from 24576 to 3072 columns per 128-row block.

Exact-match masking folds into the contraction via a 13-bit +/-1 id code:
  aug[i,col] = <q_i, d_col>                      (rows 0:32,  fp16 single term)
             + C * sum_b qbit[i,b]*dbit[col,b]   (rows 32:45, q side +/-C, doc +/-1)
             - 13*C                              (row 45: q side 1, doc -13C)
full id match    -> aug = score (code dot = +13C cancels the offset row)
id mismatch      -> aug <= score - 2C < 0   (C > max|score|, checked on host)
padding columns  -> aug = 0 (all-zero column)
so tok[i,d] = relu(max over the doc's 24 columns) reproduces the reference
exactly (the reference max includes 0 from non-matched where() zeros).

Per core: 12 fp16 matmuls [46,128]x[46,512] in 4 PSUM waves, DVE segmented
max [128,64,24]->[128,64] per wave, gpsimd relu decode, CLS scores as 12
fp16 hi/lo matmuls + per-token weighted sums as 2 fp32 matmuls, one [8,128]
output DMA.  All input DMAs ride the two hardware DGE queues (sync/scalar);
the software gpsimd queue is never on the critical path.
"""

import numpy as np
from contextlib import ExitStack

import concourse.bass as bass
import concourse.bacc as bacc
import concourse.mybir as mybir
import concourse.tile as tile
from concourse.bass_utils import run_bass_kernel_spmd

F32 = mybir.dt.float32
F16 = mybir.dt.float16

# problem shape (hardcoded per contract)
BQ, LQ, BD, LD, TOK_D, CLS_D = 64, 32, 128, 192, 32, 768
NCORES = 8
QPC = BQ // NCORES          # 8 queries per core
NBLK = 2                    # two row-blocks of 128 = 4 queries x 32 tokens
ROWS = 128
NBITS = 13                  # 2^13 = 8192 > 5000 vocab
KC = TOK_D + NBITS + 1      # 46 = embs + id code + offset row
TN = 512                    # cols per matmul = one full PSUM bank


def build_nc(cap, banks_per_wave):
    """cap = padded matched-position slots per doc; the pruned kernel uses
    cap=24 (3-bank waves, 64 docs each); the dense fallback cap=192."""
    ncol = BD * cap
    wave = banks_per_wave * TN
    assert ncol % wave == 0 and wave % cap == 0
    nwaves = ncol // wave
    dpw = wave // cap                      # docs per wave

    nc = bacc.Bacc(
        "TRN2",
        target_bir_lowering=False,
        debug=False,
        num_devices=NCORES,
    )

    # A: [qlhsT (2 blocks x 128) | pruned rhs (ncol)] fp16
    a_d = nc.dram_tensor("a", [KC, NBLK * ROWS + ncol], F16, kind="ExternalInput")
    # B: [qclsT_hi (48) | qclsT_lo (48) | dclsT_hi (768)] fp16
    b_d = nc.dram_tensor("b", [128, 2 * 48 + CLS_D], F16, kind="ExternalInput")
    # C: per-token weight selectors, fp32 (paired with fp32 tokdec matmul)
    c_d = nc.dram_tensor("c", [128, NBLK * QPC], F32, kind="ExternalInput")
    out_d = nc.dram_tensor("out", [QPC, BD], F32, kind="ExternalOutput")

    rhs0 = NBLK * ROWS                     # rhs column origin inside A

    with tile.TileContext(nc) as tc, ExitStack() as ctx:
        const = ctx.enter_context(tc.tile_pool(name="const", bufs=1))
        psum = ctx.enter_context(tc.tile_pool(name="psum", bufs=2, space="PSUM"))
        opsum = ctx.enter_context(tc.tile_pool(name="opsum", bufs=1, space="PSUM"))
        work = ctx.enter_context(tc.tile_pool(name="work", bufs=1))

        a_t = const.tile([KC, NBLK * ROWS + ncol], F16, tag="a")
        b_t = const.tile([128, 2 * 48 + CLS_D], F16, tag="b")
        c_t = const.tile([128, NBLK * QPC], F32, tag="c")

        # HW DGE queues only (sync + scalar); block-0 inputs land first
        split = rhs0 + (ncol if nwaves == 1 else nwaves // 2 * wave)
        nc.sync.dma_start(a_t[:, 0:split], a_d[:, 0:split])
        nc.sync.dma_start(a_t[:, split:], a_d[:, split:])
        nc.scalar.dma_start(b_t[:], b_d[:])
        nc.scalar.dma_start(c_t[:], c_d[:])

        out_ps = opsum.tile([QPC, BD], F32, tag="out_ps")

        # CLS scores first: PE warms up on them while rhs waves stream in
        for k in range(6):
            nc.tensor.matmul(
                out_ps[:],
                b_t[:, k * QPC:(k + 1) * QPC],
                b_t[:, 96 + k * 128:96 + (k + 1) * 128],
                start=(k == 0), stop=False,
            )
        for k in range(6):
            nc.tensor.matmul(
                out_ps[:],
                b_t[:, 48 + k * QPC:48 + (k + 1) * QPC],
                b_t[:, 96 + k * 128:96 + (k + 1) * 128],
                start=False, stop=False,
            )

        tokdec = []
        for bi in range(NBLK):
            lhs = a_t[:, bi * ROWS:(bi + 1) * ROWS]
            tokred = work.tile([ROWS, BD], F32, tag=f"tokred{bi}")
            for w in range(nwaves):
                ps = psum.tile([128, banks_per_wave, TN], F32, tag="score")
                for k in range(banks_per_wave):
                    col = rhs0 + w * wave + k * TN
                    nc.tensor.matmul(
                        ps[:, k, :], lhs, a_t[:, col:col + TN],
                        start=True, stop=True,
                    )
                red_in = ps[:, :, :].rearrange("p g t -> p (g t)").rearrange(
                    "p (d s) -> p d s", s=cap
                )
                nc.vector.reduce_max(
                    tokred[:, dpw * w:dpw * (w + 1)],
                    red_in,
                    axis=mybir.AxisListType.X,
                )
            dec = work.tile([ROWS, BD], F32, tag=f"tokdec{bi}")
            nc.gpsimd.tensor_scalar_max(dec[:], tokred[:], 0.0)
            tokdec.append(dec)

        for bi in range(NBLK):
            nc.tensor.matmul(
                out_ps[:],
                c_t[:, bi * QPC:(bi + 1) * QPC],
                tokdec[bi][:],
                start=False, stop=(bi == NBLK - 1),
            )

        outsb = work.tile([QPC, BD], F32, tag="outsb")
        nc.scalar.copy(outsb[:], out_ps[:])
        nc.sync.dma_start(out_d[:], outsb[:])

    nc.compile()
    return nc


CAP = 24
_NC_CACHE = {}


def _get_nc(cap):
    if cap not in _NC_CACHE:
        _NC_CACHE[cap] = build_nc(cap, 3)
    return _NC_CACHE[cap]


def _bits_pm1(ids):
    """ids [...] int -> [..., NBITS] float32 of +/-1 binary-code digits."""
    ids = ids.astype(np.int64)
    shifts = np.arange(NBITS, dtype=np.int64)
    return ((ids[..., None] >> shifts) & 1).astype(np.float32) * 2.0 - 1.0


def _hilo16(x):
    hi = x.astype(np.float16)
    lo = (x - hi.astype(np.float32)).astype(np.float16)
    return hi, lo


def make_in_maps(qte, dte, qce, dce, qid, did, qam):
    # SEP mask + CLS drop -> per-token weights
    sep = qam.sum(1) - 1
    qm = qam.astype(np.float32).copy()
    qm[np.arange(BQ), sep] = 0.0
    w = qm.copy()
    w[:, 0] = 0.0

    # match-bonus scale C: must exceed any |score|; L2-norm bound, fp16-exact
    bound = float(
        np.linalg.norm(qte, axis=-1).max() * np.linalg.norm(dte, axis=-1).max()
    )
    C = 96.0
    while C <= bound * 1.1:
        C *= 2.0

    qbits = _bits_pm1(qid)                        # [64, 32, 13]
    dbits_all = _bits_pm1(did)                    # [128, 192, 13]
    dte16 = dte.astype(np.float16)

    # CLS blob (shared by all cores on the doc side)
    dclsT_hi = np.ascontiguousarray(dce.T).astype(np.float16)   # [768, 128]

    # choose cap: pruned if the match pattern fits, else dense fallback
    percore_m = []
    maxpd = 0
    for c in range(NCORES):
        cq = np.unique(qid[c * QPC:(c + 1) * QPC])
        m = np.isin(did, cq)
        percore_m.append(m)
        maxpd = max(maxpd, int(m.sum(1).max()))
    cap = CAP if maxpd <= CAP else LD

    in_maps = []
    for c in range(NCORES):
        qs = slice(c * QPC, (c + 1) * QPC)
        qte_c, qbits_c, w_c = qte[qs], qbits[qs], w[qs]

        ncol = BD * cap
        a = np.zeros((KC, NBLK * ROWS + ncol), np.float16)
        for bi in range(NBLK):
            blk = qte_c[bi * 4:(bi + 1) * 4].reshape(ROWS, TOK_D)
            cols = slice(bi * ROWS, (bi + 1) * ROWS)
            a[0:TOK_D, cols] = blk.astype(np.float16).T
            a[TOK_D:TOK_D + NBITS, cols] = (
                qbits_c[bi * 4:(bi + 1) * 4].reshape(ROWS, NBITS).T * C
            )
            a[KC - 1, cols] = 1.0
        if cap == LD:
            a[0:TOK_D, NBLK * ROWS:] = dte16.transpose(2, 0, 1).reshape(TOK_D, ncol)
            a[TOK_D:TOK_D + NBITS, NBLK * ROWS:] = (
                dbits_all.transpose(2, 0, 1).reshape(NBITS, ncol)
            )
            a[KC - 1, NBLK * ROWS:] = -NBITS * C
        else:
            m = percore_m[c]
            for d in range(BD):
                js = np.nonzero(m[d])[0]
                col = NBLK * ROWS + d * cap
                e = col + len(js)
                a[0:TOK_D, col:e] = dte16[d, js].T
                a[TOK_D:TOK_D + NBITS, col:e] = dbits_all[d, js].T
                a[KC - 1, col:e] = -NBITS * C

        qcls_hi, qcls_lo = _hilo16(qce[qs])       # [8, 768] each
        b = np.zeros((128, 2 * 48 + CLS_D), np.float16)
        for k in range(6):
            ksl = slice(k * 128, (k + 1) * 128)
            b[:, k * QPC:(k + 1) * QPC] = qcls_hi.T[ksl]
            b[:, 48 + k * QPC:48 + (k + 1) * QPC] = qcls_lo.T[ksl]
            b[:, 96 + k * 128:96 + (k + 1) * 128] = dclsT_hi[ksl]

        sel = np.zeros((128, NBLK * QPC), np.float32)
        for bi in range(NBLK):
            for qq in range(4):
                ql_ = bi * 4 + qq
                sel[qq * 32:(qq + 1) * 32, bi * QPC + ql_] = w_c[ql_]

        in_maps.append({"a": a, "b": b, "c": sel})
    return in_maps, cap


def run(in_maps, cap=CAP, trace=False, **kwargs):
    nc = _get_nc(cap)
    return run_bass_kernel_spmd(
        nc, in_maps, core_ids=list(range(NCORES)), trace=trace, **kwargs
    )


def kernel(
    query_tok_embs,
    doc_tok_embs,
    query_cls_emb,
    doc_cls_emb,
    query_input_ids,
    doc_input_ids,
    query_attention_mask,
):
    qte = np.ascontiguousarray(np.asarray(query_tok_embs, np.float32))
    dte = np.ascontiguousarray(np.asarray(doc_tok_embs, np.float32))
    qce = np.ascontiguousarray(np.asarray(query_cls_emb, np.float32))
    dce = np.ascontiguousarray(np.asarray(doc_cls_emb, np.float32))
    qid = np.asarray(query_input_ids).astype(np.int64)
    did = np.asarray(doc_input_ids).astype(np.int64)
    qam = np.asarray(query_attention_mask).astype(np.int64)

    in_maps, cap = make_in_maps(qte, dte, qce, dce, qid, did, qam)
    res = run(in_maps, cap=cap)
    out = np.concatenate([r["out"] for r in res.results], axis=0)
    return np.ascontiguousarray(out.astype(np.float32))


# revision 4
# speedup vs baseline: 3.6435x; 1.0009x over previous
"""COIL-style sparse-attention scoring kernel for Trainium2 (8 NeuronCores).

Reference computation:
    scores[q,i,d,j] = <query_tok_embs[q,i], doc_tok_embs[d,j]>         (K=32)
    masked = where(query_ids[q,i]==doc_ids[d,j], scores, 0)
    tok    = masked.max(axis=j)                                        (192 -> 1)
    tok_scores[q,d] = sum_i w[q,i] * tok[q,i,d]    (w drops CLS + SEP)
    out = tok_scores + query_cls_emb @ doc_cls_emb.T

Device strategy: data-parallel over the 64 queries (8 per core).  COIL is an
inverted-list model -- a doc position (d,j) can only contribute to a query
token with the *same* token id.  With |vocab|=5000 and 256 query tokens per
core, only ~5% of the 24576 doc positions match ANY of the core's query
tokens.  The host (integer id bookkeeping only -- all float math stays on
device) keeps just those columns, padded to CAP=24 slots per doc, shrinking
the score matmul from 24576 to 3072 columns per 128-row block.

Exact-match masking folds into the contraction via a 13-bit +/-1 id code:
  aug[i,col] = <q_i, d_col>                      (rows 0:32,  fp16 single term)
             + C * sum_b qbit[i,b]*dbit[col,b]   (rows 32:45, q side +/-C, doc +/-1)
             - 13*C                              (row 45: q side 1, doc -13C)
full id match    -> aug = score (code dot = +13C cancels the offset row)
id mismatch      -> aug <= score - 2C < 0   (C > max|score|, checked on host)
padding columns  -> aug = 0 (all-zero column)
so tok[i,d] = relu(max over the doc's 24 columns) reproduces the reference
exactly (the reference max includes 0 from non-matched where() zeros).

Per core: 12 fp16 matmuls [46,128]x[46,512] in 4 PSUM waves, DVE segmented
max [128,64,24]->[128,64] per wave, gpsimd relu decode, CLS scores as 12
fp16 hi/lo matmuls + per-token weighted sums as 2 fp32 matmuls, one [8,128]
output DMA.  All input DMAs ride the two hardware DGE queues (sync/scalar);
the software gpsimd queue is never on the critical path.
"""

import numpy as np
from contextlib import ExitStack

import concourse.bass as bass
import concourse.bacc as bacc
import concourse.mybir as mybir
import concourse.tile as tile
from concourse.bass_utils import run_bass_kernel_spmd

F32 = mybir.dt.float32
F16 = mybir.dt.float16

# problem shape (hardcoded per contract)
BQ, LQ, BD, LD, TOK_D, CLS_D = 64, 32, 128, 192, 32, 768
NCORES = 8
QPC = BQ // NCORES          # 8 queries per core
NBLK = 2                    # two row-blocks of 128 = 4 queries x 32 tokens
ROWS = 128
NBITS = 13                  # 2^13 = 8192 > 5000 vocab
KC = TOK_D + NBITS + 1      # 46 = embs + id code + offset row
TN = 512                    # cols per matmul = one full PSUM bank


def build_nc(cap, banks_per_wave):
    """cap = padded matched-position slots per doc; the pruned kernel uses
    cap=24 (3-bank waves, 64 docs each); the dense fallback cap=192."""
    ncol = BD * cap
    wave = banks_per_wave * TN
    assert ncol % wave == 0 and wave % cap == 0
    nwaves = ncol // wave
    dpw = wave // cap                      # docs per wave

    nc = bacc.Bacc(
        "TRN2",
        target_bir_lowering=False,
        debug=False,
        num_devices=NCORES,
    )

    # A: [qlhsT (2 blocks x 128) | pruned rhs (ncol)] fp16
    a_d = nc.dram_tensor("a", [KC, NBLK * ROWS + ncol], F16, kind="ExternalInput")
    # B: [qclsT_hi (48) | qclsT_lo (48) | dclsT_hi (768)] fp16
    b_d = nc.dram_tensor("b", [128, 2 * 48 + CLS_D], F16, kind="ExternalInput")
    # C: per-token weight selectors, fp32 (paired with fp32 tokdec matmul)
    c_d = nc.dram_tensor("c", [128, NBLK * QPC], F32, kind="ExternalInput")
    out_d = nc.dram_tensor("out", [QPC, BD], F32, kind="ExternalOutput")

    rhs0 = NBLK * ROWS                     # rhs column origin inside A

    with tile.TileContext(nc) as tc, ExitStack() as ctx:
        const = ctx.enter_context(tc.tile_pool(name="const", bufs=1))
        psum = ctx.enter_context(tc.tile_pool(name="psum", bufs=2, space="PSUM"))
        opsum = ctx.enter_context(tc.tile_pool(name="opsum", bufs=1, space="PSUM"))
        work = ctx.enter_context(tc.tile_pool(name="work", bufs=1))

        a_t = const.tile([KC, NBLK * ROWS + ncol], F16, tag="a")
        b_t = const.tile([128, 2 * 48 + CLS_D], F16, tag="b")
        c_t = const.tile([128, NBLK * QPC], F32, tag="c")

        # HW DGE queues only.  sync fans out over ~14 DMA queues (fast),
        # scalar over 2 (slow) -> early-needed data on sync, tail on scalar.
        split = rhs0 + (ncol if nwaves == 1 else (nwaves + 1) // 2 * wave)
        nc.sync.dma_start(a_t[:, 0:split], a_d[:, 0:split])
        nc.sync.dma_start(b_t[:], b_d[:])
        nc.scalar.dma_start(a_t[:, split:], a_d[:, split:])
        nc.scalar.dma_start(c_t[:], c_d[:])

        out_ps = opsum.tile([QPC, BD], F32, tag="out_ps")

        def cls_matmuls(first):
            for k in range(6):
                nc.tensor.matmul(
                    out_ps[:],
                    b_t[:, k * QPC:(k + 1) * QPC],
                    b_t[:, 96 + k * 128:96 + (k + 1) * 128],
                    start=(first and k == 0), stop=False,
                )
            for k in range(6):
                nc.tensor.matmul(
                    out_ps[:],
                    b_t[:, 48 + k * QPC:48 + (k + 1) * QPC],
                    b_t[:, 96 + k * 128:96 + (k + 1) * 128],
                    start=False, stop=False,
                )

        # out_ps accumulation group opens with the CLS matmuls (they are
        # emitted after wave 0/1 so the PE starts on blob A, which lands
        # first; B only gates the CLS block)
        tokdec = []
        tokreds = []
        cls_done = False
        for bi in range(NBLK):
            lhs = a_t[:, bi * ROWS:(bi + 1) * ROWS]
            tokred = work.tile([ROWS, BD], F32, tag=f"tokred{bi}")
            tokreds.append(tokred)
            for w in range(nwaves):
                ps = psum.tile([128, banks_per_wave, TN], F32, tag="score")
                for k in range(banks_per_wave):
                    col = rhs0 + w * wave + k * TN
                    nc.tensor.matmul(
                        ps[:, k, :], lhs, a_t[:, col:col + TN],
                        start=True, stop=True,
                    )
                if bi == 0 and w == (2 if nwaves > 2 else nwaves) - 1 and not cls_done:
                    cls_matmuls(first=True)
                    cls_done = True
                # reduce in halves so each half starts as soon as its
                # matmuls land (shorter pipeline + shorter tail)
                flat = ps[:, :, :].rearrange("p g t -> p (g t)")
                half = wave // 2
                for h in range(2):
                    red_in = flat[:, h * half:(h + 1) * half].rearrange(
                        "p (d s) -> p d s", s=cap
                    )
                    nc.vector.reduce_max(
                        tokred[:, dpw * w + dpw // 2 * h:
                               dpw * w + dpw // 2 * (h + 1)],
                        red_in,
                        axis=mybir.AxisListType.X,
                    )
        if not cls_done:
            cls_matmuls(first=True)

        for bi in range(NBLK):
            dec = work.tile([ROWS, BD], F32, tag=f"tokdec{bi}")
            nc.scalar.activation(
                dec[:], tokreds[bi][:], mybir.ActivationFunctionType.Relu,
            )
            tokdec.append(dec)

        for bi in range(NBLK):
            nc.tensor.matmul(
                out_ps[:],
                c_t[:, bi * QPC:(bi + 1) * QPC],
                tokdec[bi][:],
                start=False, stop=(bi == NBLK - 1),
            )

        outsb = work.tile([QPC, BD], F32, tag="outsb")
        nc.scalar.copy(outsb[:], out_ps[:])
        nc.sync.dma_start(out_d[:], outsb[:])

    nc.compile()
    return nc


CAP = 24
_NC_CACHE = {}


def _get_nc(cap):
    if cap not in _NC_CACHE:
        _NC_CACHE[cap] = build_nc(cap, 3)
    return _NC_CACHE[cap]


def _bits_pm1(ids):
    """ids [...] int -> [..., NBITS] float32 of +/-1 binary-code digits."""
    ids = ids.astype(np.int64)
    shifts = np.arange(NBITS, dtype=np.int64)
    return ((ids[..., None] >> shifts) & 1).astype(np.float32) * 2.0 - 1.0


def _hilo16(x):
    hi = x.astype(np.float16)
    lo = (x - hi.astype(np.float32)).astype(np.float16)
    return hi, lo


def make_in_maps(qte, dte, qce, dce, qid, did, qam):
    # SEP mask + CLS drop -> per-token weights
    sep = qam.sum(1) - 1
    qm = qam.astype(np.float32).copy()
    qm[np.arange(BQ), sep] = 0.0
    w = qm.copy()
    w[:, 0] = 0.0

    # match-bonus scale C: must exceed any |score|; L2-norm bound, fp16-exact
    bound = float(
        np.linalg.norm(qte, axis=-1).max() * np.linalg.norm(dte, axis=-1).max()
    )
    C = 96.0
    while C <= bound * 1.1:
        C *= 2.0

    qbits = _bits_pm1(qid)                        # [64, 32, 13]
    dbits_all = _bits_pm1(did)                    # [128, 192, 13]
    dte16 = dte.astype(np.float16)

    # CLS blob (shared by all cores on the doc side)
    dclsT_hi = np.ascontiguousarray(dce.T).astype(np.float16)   # [768, 128]

    # choose cap: pruned if the match pattern fits, else dense fallback
    percore_m = []
    maxpd = 0
    for c in range(NCORES):
        cq = np.unique(qid[c * QPC:(c + 1) * QPC])
        m = np.isin(did, cq)
        percore_m.append(m)
        maxpd = max(maxpd, int(m.sum(1).max()))
    cap = CAP if maxpd <= CAP else LD

    in_maps = []
    for c in range(NCORES):
        qs = slice(c * QPC, (c + 1) * QPC)
        qte_c, qbits_c, w_c = qte[qs], qbits[qs], w[qs]

        ncol = BD * cap
        a = np.zeros((KC, NBLK * ROWS + ncol), np.float16)
        for bi in range(NBLK):
            blk = qte_c[bi * 4:(bi + 1) * 4].reshape(ROWS, TOK_D)
            cols = slice(bi * ROWS, (bi + 1) * ROWS)
            a[0:TOK_D, cols] = blk.astype(np.float16).T
            a[TOK_D:TOK_D + NBITS, cols] = (
                qbits_c[bi * 4:(bi + 1) * 4].reshape(ROWS, NBITS).T * C
            )
            a[KC - 1, cols] = 1.0
        if cap == LD:
            a[0:TOK_D, NBLK * ROWS:] = dte16.transpose(2, 0, 1).reshape(TOK_D, ncol)
            a[TOK_D:TOK_D + NBITS, NBLK * ROWS:] = (
                dbits_all.transpose(2, 0, 1).reshape(NBITS, ncol)
            )
            a[KC - 1, NBLK * ROWS:] = -NBITS * C
        else:
            m = percore_m[c]
            for d in range(BD):
                js = np.nonzero(m[d])[0]
                col = NBLK * ROWS + d * cap
                e = col + len(js)
                a[0:TOK_D, col:e] = dte16[d, js].T
                a[TOK_D:TOK_D + NBITS, col:e] = dbits_all[d, js].T
                a[KC - 1, col:e] = -NBITS * C

        qcls_hi, qcls_lo = _hilo16(qce[qs])       # [8, 768] each
        b = np.zeros((128, 2 * 48 + CLS_D), np.float16)
        for k in range(6):
            ksl = slice(k * 128, (k + 1) * 128)
            b[:, k * QPC:(k + 1) * QPC] = qcls_hi.T[ksl]
            b[:, 48 + k * QPC:48 + (k + 1) * QPC] = qcls_lo.T[ksl]
            b[:, 96 + k * 128:96 + (k + 1) * 128] = dclsT_hi[ksl]

        sel = np.zeros((128, NBLK * QPC), np.float32)
        for bi in range(NBLK):
            for qq in range(4):
                ql_ = bi * 4 + qq
                sel[qq * 32:(qq + 1) * 32, bi * QPC + ql_] = w_c[ql_]

        in_maps.append({"a": a, "b": b, "c": sel})
    return in_maps, cap


def run(in_maps, cap=CAP, trace=False, **kwargs):
    nc = _get_nc(cap)
    return run_bass_kernel_spmd(
        nc, in_maps, core_ids=list(range(NCORES)), trace=trace, **kwargs
    )


def kernel(
    query_tok_embs,
    doc_tok_embs,
    query_cls_emb,
    doc_cls_emb,
    query_input_ids,
    doc_input_ids,
    query_attention_mask,
):
    qte = np.ascontiguousarray(np.asarray(query_tok_embs, np.float32))
    dte = np.ascontiguousarray(np.asarray(doc_tok_embs, np.float32))
    qce = np.ascontiguousarray(np.asarray(query_cls_emb, np.float32))
    dce = np.ascontiguousarray(np.asarray(doc_cls_emb, np.float32))
    qid = np.asarray(query_input_ids).astype(np.int64)
    did = np.asarray(doc_input_ids).astype(np.int64)
    qam = np.asarray(query_attention_mask).astype(np.int64)

    in_maps, cap = make_in_maps(qte, dte, qce, dce, qid, did, qam)
    res = run(in_maps, cap=cap)
    out = np.concatenate([r["out"] for r in res.results], axis=0)
    return np.ascontiguousarray(out.astype(np.float32))
